# revision 13
# baseline (speedup 1.0000x reference)
"""Trainium2 Bass kernel for causal multi-head attention with adaptive
temperature (entropy-polynomial) softmax.

Problem shape: x [2, 2048, 1024], 16 heads x 64 dims, causal.
  q/k/v = x @ W{q,k,v}.T ; sim = q k^T / 8 (causal) ;
  attn = softmax(beta * sim), beta = f(entropy(softmax(sim))) ;
  out = (attn v) @ Wo.T + bo

Sharding (8 cores): core c owns batch b = c // 4 and heads
4*(c%4) .. 4*(c%4)+3.  Host sums the 4 partials per batch and adds bo.

Device-side per core (m indexes head pairs {2m, 2m+1}):
  phase A   : qT/kT = (Wq/Wk slice) @ x^T (f32r, qT pre-scaled 1/8),
              kT also cast fp16; v packed fp16 + ones column per head
  B1(m)     : entropy-stat sweep in [i, j]: scores -> exp (accum Z1)
              -> l*exp(l) (accum D)
  stats(m)  : H = ln Z1 - D/Z1 ; beta = where(H>.5, max(poly(H),1), 1);
              fold beta into q: qbT = qT * beta (fp16) via PE transpose +
              SBUF DMA gather + partition_broadcast
  B2(m)     : transposed pass: scoresT[j,i] = kT16^T @ qbT, maskT on
              diagonal, exp -> e2T fp16, avT[65,512] += v_aug^T @ e2T
              (row 64 = Z2 via ones col), normalize by 1/Z2 -> attT
  phase C   : partial = attT.T @ Wo_slice^T (f32r)

Pipeline order A; B1(0); stats(0); B1(1); stats(1); B2(0); B2(1); C
keeps the PE busy through the stats/beta-fold serial chain.
"""

import numpy as np

import concourse.bass as bass
import concourse.tile as tile
from concourse import bacc, mybir
from concourse.bass_utils import run_bass_kernel_spmd
from concourse.masks import make_identity

F32 = mybir.dt.float32
F32R = mybir.dt.float32r
BF16 = mybir.dt.bfloat16
FP16 = mybir.dt.float16
I32 = mybir.dt.int32
AFT = mybir.ActivationFunctionType
ALU = mybir.AluOpType

B, N, DIM = 2, 2048, 1024
H_TOT, HD = 16, 64
N_CORES = 8
NH = 4            # heads per core
CD = NH * HD      # 256 channel dims per core
NRB = N // 128    # 16 row blocks
NG = NRB // 4     # 4 groups of 4 row blocks (512 query cols each)
POLY = [-0.037, 0.481, -2.3, 4.917, -1.791]
MASK_VAL = -1e30
SCALE = 1.0 / 8.0  # 1/sqrt(64)

CHUNK = 512          # B1 score-chunk width (PSUM tile free size)
NUC = NRB * 2        # 32 (rb, head-in-pair) units per m


def _scores(nc, ps, q_l, kTm, base, off, cw):
    for o2 in range(0, cw, 512):
        sw = min(512, cw - o2)
        nc.tensor.matmul(ps[:, o2:o2 + sw], q_l,
                         kTm[base:base + 64, off + o2:off + o2 + sw],
                         start=True, stop=True)


def build_kernel():
    nc = bacc.Bacc("TRN2", target_bir_lowering=False, debug=False,
                   num_devices=N_CORES)

    xT = nc.dram_tensor("xT", [DIM, N], F32, kind="ExternalInput").ap()
    wqT = nc.dram_tensor("wqT", [DIM, CD], F32, kind="ExternalInput").ap()
    wkT = nc.dram_tensor("wkT", [DIM, CD], F32, kind="ExternalInput").ap()
    wvT = nc.dram_tensor("wvT", [DIM, CD], F32, kind="ExternalInput").ap()
    woT = nc.dram_tensor("woT", [CD, DIM], F32, kind="ExternalInput").ap()
    maskin = nc.dram_tensor("maskin", [128, 128], F32, kind="ExternalInput").ap()
    maskTin = nc.dram_tensor("maskTin", [128, 128], F32, kind="ExternalInput").ap()
    partial = nc.dram_tensor("partial", [N, DIM], F32, kind="ExternalOutput").ap()

    KC = DIM // 128  # 8 contraction chunks

    with tile.TileContext(nc) as tc:
        with tc.tile_pool(name="const", bufs=1) as constp, \
             tc.tile_pool(name="qkv_sb", bufs=1) as qkvp, \
             tc.tile_pool(name="attn_out", bufs=1) as aop, \
             tc.tile_pool(name="wo_sb", bufs=1) as wop, \
             tc.tile_pool(name="statsall", bufs=1) as sap:

            identf = constp.tile([128, 128], F32)
            make_identity(nc, identf[:])
            mask = constp.tile([128, 128], F32)
            nc.sync.dma_start(mask[:], maskin[:])
            maskT = constp.tile([128, 128], F32)
            nc.sync.dma_start(maskT[:], maskTin[:])
            ones32 = constp.tile([128, NUC], F32)
            nc.vector.memset(ones32[:], 1.0)

            qT = [qkvp.tile([128, N], F32R, tag=f"qT{m}", name=f"qT{m}") for m in range(2)]
            kT = [qkvp.tile([128, N], F32R, tag=f"kT{m}", name=f"kT{m}") for m in range(2)]
            kT16 = [qkvp.tile([128, N], FP16, tag=f"kT16{m}", name=f"kT16{m}") for m in range(2)]
            vaug = [qkvp.tile([128, NH * 65], FP16, tag=f"va{j}", name=f"va{j}") for j in range(NRB)]
            qbT = [qkvp.tile([128, N], FP16, tag=f"qb{m}", name=f"qb{m}") for m in range(2)]
            attT = [aop.tile([128, N], F32R, tag=f"attT{m}", name=f"attT{m}") for m in range(2)]
            woS = [wop.tile([128, DIM], F32R, tag=f"wo{m}", name=f"wo{m}") for m in range(2)]

            # per-m stats accumulators: col = (rb*2 + hh)*4 + chunk
            Z1p = [sap.tile([128, 4 * NUC], F32, tag=f"Z1p{m}", name=f"Z1p{m}") for m in range(2)]
            D1p = [sap.tile([128, 4 * NUC], F32, tag=f"D1p{m}", name=f"D1p{m}") for m in range(2)]

            # ---- phase A: QKV projections ----
            with tc.tile_pool(name="xw_sb", bufs=1) as xwp, \
                 tc.tile_pool(name="qkv_ps", bufs=4, space="PSUM") as qkps:
                xTs = [xwp.tile([128, N], F32R, tag=f"xT{k}", name=f"xTs{k}") for k in range(KC)]
                wq_s = [xwp.tile([128, CD], F32R, tag=f"wq{k}", name=f"wq{k}") for k in range(KC)]
                wk_s = [xwp.tile([128, CD], F32R, tag=f"wk{k}", name=f"wk{k}") for k in range(KC)]
                wv_s = [xwp.tile([128, CD], F32R, tag=f"wv{k}", name=f"wv{k}") for k in range(KC)]
                for k in range(KC):
                    sl = slice(128 * k, 128 * (k + 1))
                    nc.sync.dma_start(wq_s[k][:], wqT[sl, :].bitcast(F32R))
                    nc.sync.dma_start(wk_s[k][:], wkT[sl, :].bitcast(F32R))
                    nc.sync.dma_start(wv_s[k][:], wvT[sl, :].bitcast(F32R))
                    nc.sync.dma_start(xTs[k][:], xT[sl, :].bitcast(F32R))
                for m in range(2):
                    nc.sync.dma_start(woS[m][:], woT[128 * m:128 * (m + 1), :].bitcast(F32R))

                for m in range(2):
                    for which, wt, dest, scl in (("q", wq_s, qT, SCALE), ("k", wk_s, kT, 1.0)):
                        for nn in range(N // 512):
                            pq = qkps.tile([128, 512], F32, tag="pq")
                            for k in range(KC):
                                nc.tensor.matmul(
                                    pq[:], wt[k][:, 128 * m:128 * (m + 1)],
                                    xTs[k][:, 512 * nn:512 * (nn + 1)],
                                    start=(k == 0), stop=(k == KC - 1))
                            nc.scalar.activation(
                                dest[m][:, 512 * nn:512 * (nn + 1)], pq[:],
                                AFT.Copy, bias=0.0, scale=scl)
                            if which == "k":
                                nc.vector.tensor_copy(
                                    kT16[m][:, 512 * nn:512 * (nn + 1)], pq[:])

                for jt in range(NRB):
                    pv = qkps.tile([128, CD], F32, tag="pv")
                    for k in range(KC):
                        nc.tensor.matmul(
                            pv[:], xTs[k][:, 128 * jt:128 * (jt + 1)], wv_s[k][:],
                            start=(k == 0), stop=(k == KC - 1))
                    va3 = vaug[jt][:].rearrange("p (h x) -> p h x", x=65)
                    nc.any.tensor_copy(
                        va3[:, :, 0:64],
                        pv[:].rearrange("p (h x) -> p h x", x=64))
                    nc.gpsimd.memset(va3[:, :, 64:65], 1.0)

            # ---- pipelined B1 / stats / B2 ----
            with tc.tile_pool(name="scr", bufs=6) as scrp, \
                 tc.tile_pool(name="stats", bufs=2) as stp, \
                 tc.tile_pool(name="bprep", bufs=2) as bpp, \
                 tc.tile_pool(name="e2sb", bufs=6) as e2p, \
                 tc.tile_pool(name="b1_ps", bufs=2, space="PSUM") as scps, \
                 tc.tile_pool(name="bt_ps", bufs=1, space="PSUM") as btps, \
                 tc.tile_pool(name="b2_ps", bufs=3, space="PSUM") as b2ps, \
                 tc.tile_pool(name="av_ps", bufs=2, space="PSUM") as avps:

                def b1_sweep(m):
                    nc.vector.memset(Z1p[m][:], 0.0)
                    nc.vector.memset(D1p[m][:], 0.0)
                    for rb in range(NRB):
                        W = 128 * (rb + 1)
                        chunks = [(off, min(CHUNK, W - off)) for off in range(0, W, CHUNK)]
                        for hh in range(2):
                            base = 64 * hh
                            col = (rb * 2 + hh) * 4
                            q_l = qT[m][base:base + 64, 128 * rb:128 * (rb + 1)]
                            for ci, (off, cw) in enumerate(chunks):
                                ps = scps.tile([128, CHUNK], F32, tag="ps_s")
                                _scores(nc, ps, q_l, kT[m], base, off, cw)
                                if off + cw == W:
                                    nc.vector.tensor_tensor(
                                        out=ps[:, cw - 128:cw], in0=ps[:, cw - 128:cw],
                                        in1=mask[:], op=ALU.add)
                                t1 = scrp.tile([128, CHUNK], F32, tag="t1")
                                nc.scalar.activation(
                                    t1[:, :cw], ps[:, :cw], AFT.Exp,
                                    bias=0.0, scale=1.0,
                                    accum_out=Z1p[m][:, col + ci:col + ci + 1])
                                s2 = scrp.tile([128, CHUNK], F32, tag="s2")
                                nc.vector.scalar_tensor_tensor(
                                    out=s2[:, :cw], in0=ps[:, :cw], scalar=1.0,
                                    in1=t1[:, :cw], op0=ALU.mult, op1=ALU.mult,
                                    accum_out=D1p[m][:, col + ci:col + ci + 1])

                def stats_and_fold(m):
                    Z1a = stp.tile([128, NUC], F32, tag="Z1a")
                    D1a = stp.tile([128, NUC], F32, tag="D1a")
                    nc.vector.tensor_reduce(
                        out=Z1a[:], in_=Z1p[m].rearrange("p (u c) -> p u c", c=4),
                        axis=mybir.AxisListType.X, op=ALU.add)
                    nc.vector.tensor_reduce(
                        out=D1a[:], in_=D1p[m].rearrange("p (u c) -> p u c", c=4),
                        axis=mybir.AxisListType.X, op=ALU.add)
                    rz = stp.tile([128, NUC], F32, tag="rz")
                    nc.vector.reciprocal(rz[:], Z1a[:])
                    dn = stp.tile([128, NUC], F32, tag="dn")
                    nc.vector.tensor_mul(dn[:], D1a[:], rz[:])
                    lnz = stp.tile([128, NUC], F32, tag="lnz")
                    nc.scalar.activation(lnz[:], Z1a[:], AFT.Ln, bias=0.0, scale=1.0)
                    Hent = stp.tile([128, NUC], F32, tag="Hent")
                    nc.vector.tensor_sub(Hent[:], lnz[:], dn[:])
                    p0 = stp.tile([128, NUC], F32, tag="p0")
                    nc.vector.tensor_scalar(out=p0[:], in0=Hent[:], scalar1=POLY[0],
                                            scalar2=POLY[1], op0=ALU.mult, op1=ALU.add)
                    p1 = stp.tile([128, NUC], F32, tag="p1")
                    for c in POLY[2:]:
                        nc.vector.tensor_mul(p1[:], p0[:], Hent[:])
                        nc.vector.tensor_scalar_add(p0[:], p1[:], c)
                    nc.vector.tensor_scalar_max(p1[:], p0[:], 1.0)
                    mk = stp.tile([128, NUC], I32, tag="mk")
                    nc.vector.tensor_scalar(out=mk[:], in0=Hent[:], scalar1=0.5,
                                            scalar2=None, op0=ALU.is_gt)
                    beta_m = stp.tile([128, NUC], F32, tag="beta_m")
                    nc.vector.tensor_copy(beta_m[:], ones32[:])
                    nc.vector.copy_predicated(beta_m[:], mk[:], p1[:])

                    # fold beta into q (fp16): transpose beta -> row per head
                    # -> partition broadcast -> multiply
                    btp = btps.tile([NUC, 128], F32, tag="btp")
                    nc.tensor.transpose(btp[:], beta_m[:], identf[:])
                    betaT = bpp.tile([NUC, 128], F32, tag="betaT")
                    nc.any.tensor_copy(betaT[:], btp[:])
                    for hh in range(2):
                        base = 64 * hh
                        brow = bpp.tile([1, N], F32, tag="brow")
                        nc.sync.dma_start(brow[:], betaT[hh::2, :])
                        bb = bpp.tile([128, N], F32, tag="bb")
                        nc.gpsimd.partition_broadcast(bb[:], brow[:])
                        nc.vector.tensor_tensor(
                            out=qbT[m][base:base + 64, :],
                            in0=qT[m][base:base + 64, :].bitcast(F32),
                            in1=bb[base:base + 64, :], op=ALU.mult)

                def b2_sweep(m):
                    for hh in range(2):
                        h = 2 * m + hh
                        base = 64 * hh
                        for g in range(NG):
                            i0 = 512 * g
                            njt = 4 * g + 4
                            avp = avps.tile([65, 512], F32, tag="avp")
                            for b0 in range(0, njt, 4):
                                bjts = range(b0, min(b0 + 4, njt))
                                e2s = {}
                                for jt in bjts:
                                    off = max(0, 128 * (jt - 4 * g))
                                    w = 512 - off
                                    ps2 = b2ps.tile([128, 512], F32, tag="ps2")
                                    nc.tensor.matmul(
                                        ps2[:, 0:w],
                                        kT16[m][base:base + 64, 128 * jt:128 * (jt + 1)],
                                        qbT[m][base:base + 64, i0 + off:i0 + 512],
                                        start=True, stop=True)
                                    if jt >= 4 * g:
                                        nc.vector.tensor_tensor(
                                            out=ps2[:, 0:128], in0=ps2[:, 0:128],
                                            in1=maskT[:], op=ALU.add)
                                    e2 = e2p.tile([128, 512], FP16, tag="e2")
                                    nc.scalar.activation(
                                        e2[:, 0:w], ps2[:, 0:w], AFT.Exp,
                                        bias=0.0, scale=1.0)
                                    e2s[jt] = (e2, off, w)
                                for jt in bjts:
                                    e2, off, w = e2s[jt]
                                    nc.tensor.matmul(
                                        avp[:, off:512],
                                        vaug[jt][:, 65 * h:65 * h + 65],
                                        e2[:, 0:w],
                                        start=(jt == 0), stop=(jt == njt - 1),
                                        skip_group_check=True)
                            zrow = stp.tile([1, 512], F32, tag="zrow")
                            nc.vector.reciprocal(zrow[:], avp[64:65, :])
                            rbv = stp.tile([64, 512], F32, tag="rbv")
                            nc.gpsimd.partition_broadcast(rbv[:], zrow[:])
                            nc.vector.tensor_tensor(
                                out=attT[m][base:base + 64, i0:i0 + 512],
                                in0=avp[0:64, :], in1=rbv[:], op=ALU.mult)

                b1_sweep(0)
                stats_and_fold(0)
                b1_sweep(1)
                stats_and_fold(1)
                b2_sweep(0)
                b2_sweep(1)

            # ---- phase C: output projection ----
            with tc.tile_pool(name="ost", bufs=3) as ostp, \
                 tc.tile_pool(name="pj_ps", bufs=2, space="PSUM") as pjps:
                for rb in range(NRB):
                    for nn in range(2):
                        pp = pjps.tile([128, 512], F32, tag="pp")
                        for m in range(2):
                            nc.tensor.matmul(
                                pp[:], attT[m][:, 128 * rb:128 * (rb + 1)],
                                woS[m][:, 512 * nn:512 * (nn + 1)],
                                start=(m == 0), stop=(m == 1))
                        ost = ostp.tile([128, 512], F32, tag="ost")
                        nc.vector.tensor_copy(ost[:], pp[:])
                        nc.sync.dma_start(
                            partial[128 * rb:128 * (rb + 1), 512 * nn:512 * (nn + 1)],
                            ost[:])

    nc.compile()
    return nc


_NC_CACHE = None
_LAST_IN_MAPS = None


def kernel(x, Wq, Wk, Wv, Wo, bo):
    global _NC_CACHE, _LAST_IN_MAPS
    x = np.asarray(x, dtype=np.float32)
    Wq = np.asarray(Wq, dtype=np.float32)
    Wk = np.asarray(Wk, dtype=np.float32)
    Wv = np.asarray(Wv, dtype=np.float32)
    Wo = np.asarray(Wo, dtype=np.float32)
    bo = np.asarray(bo, dtype=np.float32)

    if _NC_CACHE is None:
        _NC_CACHE = build_kernel()
    nc = _NC_CACHE

    mask_h = np.where(np.arange(128)[None, :] > np.arange(128)[:, None],
                      np.float32(MASK_VAL), np.float32(0.0)).astype(np.float32)
    maskT_h = np.ascontiguousarray(mask_h.T)
    woT_full = np.ascontiguousarray(Wo.T)  # [c, o]

    in_maps = []
    for c in range(N_CORES):
        b = c // 4
        s0 = CD * (c % 4)
        sl = slice(s0, s0 + CD)
        in_maps.append({
            "xT": np.ascontiguousarray(x[b].T),
            "wqT": np.ascontiguousarray(Wq[sl, :].T),
            "wkT": np.ascontiguousarray(Wk[sl, :].T),
            "wvT": np.ascontiguousarray(Wv[sl, :].T),
            "woT": np.ascontiguousarray(woT_full[sl, :]),
            "maskin": mask_h,
            "maskTin": maskT_h,
        })

    _LAST_IN_MAPS = in_maps
    res = run_bass_kernel_spmd(nc, in_maps, core_ids=list(range(N_CORES)))

    out = np.zeros((B, N, DIM), dtype=np.float32)
    for c in range(N_CORES):
        out[c // 4] += res.results[c]["partial"]
    out += bo[None, None, :]
    return out
